# revision 11
# baseline (speedup 1.0000x reference)
"""Trainium2 Bass kernel for nn_MessagePassingLayer (gnn_message_passing).

Sharding: batch (B=8) across 8 NeuronCores, one batch element per core.

Per-core layout: E = A*N = 16384 edges in 128 chunks of 128 edges (2 atoms).
  R-form: [128 edges (partitions), F]   -- gathers, edge-contraction matmuls
  T-form: [128 F (partitions), edges]   -- feature-contraction matmuls
Neighbor segment sums are PE matmuls contracting the edge partition dim
against small per-chunk weight blocks (mask / m*dv*cF, blocked by atom
parity), producing feature-major (T-form) results packed into PSUM and
drained in bulk. The polynomial cutoff c(d) is folded into the
node-feature-replication matmul (its stationary parity pattern is c-scaled).
imsg is transposed R->T once via PE transposes.
"""
import numpy as np
import ml_dtypes

import concourse.bass as bass
import concourse.bacc as bacc
import concourse.tile as tile
from concourse import mybir
from concourse.bass import IndirectOffsetOnAxis
from concourse.masks import make_identity

F32 = mybir.dt.float32
BF16 = mybir.dt.bfloat16
I32 = mybir.dt.int32

B, A, N, F, NB = 8, 256, 64, 128, 20
E = A * N
NCH = E // 128            # 128 chunks
GRP = 16                  # chunks per group
NG = NCH // GRP
CUTOFF, P_EXP = 5.0, 9.0

AluOp = mybir.AluOpType
ActFn = mybir.ActivationFunctionType
BFNP = ml_dtypes.bfloat16


def _ap(t, off, pattern):
    """AP over tile t keeping its partition pair, custom free pattern."""
    return bass.AP(t.tensor, t.offset + off, [list(t.ap[0])] + [list(p) for p in pattern])


def build_program(use_bf1=False, use_bf2=False):
    nc = bacc.Bacc("TRN2", target_bir_lowering=False, debug=False, num_devices=8)

    def din(name, shape, dt):
        return nc.dram_tensor(name, shape, dt, kind="ExternalInput")

    d_edgeT = din("edgeT", [NB + 1, E], BF16)
    d_drtab = din("drtab", [A, 3 * F], BF16)
    d_nbr = din("nbr", [128, NCH], I32)
    d_dist = din("dist", [128, NCH], F32)
    d_mask = din("mask", [128, NCH], F32)
    d_dv = din("dv", [128, 3 * NCH], F32)
    d_invT = din("invT", [F, A], F32)
    d_eqFh = din("eqFh", [2, 3 * NCH], F32)
    d_eqfT = din("eqfT", [F, 6 * NCH], F32)     # [f, c*6+d*2+a2]
    d_eqdrT = din("eqdrT", [F, 6 * NCH], F32)
    d_wme = din("wme", [NB + 1, F], BF16)
    d_wf1c = din("wf1c", [F, F + 1], BF16)
    d_we1 = din("we1", [F, F], BF16)
    d_we2 = din("we2", [F, F], BF16)
    d_blkm_b = din("blkm_b", [128, 2], BF16)
    d_blkm_f = din("blkm_f", [128, 2], F32)
    d_wn1 = din("wn1", [F, F], F32)
    d_wn2 = din("wn2", [F, F], F32)
    d_wf2 = din("wf2", [F, F], F32)
    d_ws1 = din("ws1", [F, F], F32)
    d_ws2 = din("ws2", [F, F], F32)
    d_wi1 = din("wi1", [F, F], F32)
    d_wi2 = din("wi2", [F, F], F32)
    d_bias = din("biasv", [F, 6], F32)          # bn1,bn2,bs1,bs2,bi1,bi2
    d_bf1 = din("bf1bc", [128, F + 1], F32)
    d_bf2 = din("bf2row", [1, F], F32)

    o_inv = nc.dram_tensor("o_inv", [A, F], F32, kind="ExternalOutput")
    o_eqF = nc.dram_tensor("o_eqF", [A, 3], F32, kind="ExternalOutput")
    o_eqf = nc.dram_tensor("o_eqf", [A, 3 * F], F32, kind="ExternalOutput")
    o_eqdr = nc.dram_tensor("o_eqdr", [A, 3 * F], F32, kind="ExternalOutput")
    gtab = nc.dram_tensor("gtab", [A, 4 * F], BF16)

    with tile.TileContext(nc) as tc:
        with tc.tile_pool(name="cst", bufs=1) as cst, \
             tc.tile_pool(name="ps", bufs=8, space="PSUM") as ps, \
             tc.tile_pool(name="gathp", bufs=2) as gathp, \
             tc.tile_pool(name="wrk", bufs=2) as wrk, \
             tc.tile_pool(name="wrk1", bufs=1) as wrk1, \
             tc.tile_pool(name="fin", bufs=1) as fin:

            def load(d, shape, dt):
                t = cst.tile(shape, dt, name=d.name + "_sb")
                nc.sync.dma_start(out=t[:], in_=d[tuple(slice(None) for _ in shape)])
                return t

            nbr_sb = load(d_nbr, [128, NCH], I32)
            dist = load(d_dist, [128, NCH], F32)
            m_pc = load(d_mask, [128, NCH], F32)
            dv_pc = load(d_dv, [128, 3 * NCH], F32)
            invT = load(d_invT, [F, A], F32)
            eqFh = load(d_eqFh, [2, 3 * NCH], F32)
            eqfT = load(d_eqfT, [F, 6 * NCH], F32)
            eqdrT = load(d_eqdrT, [F, 6 * NCH], F32)
            wme = load(d_wme, [NB + 1, F], BF16)
            wf1c = load(d_wf1c, [F, F + 1], BF16)
            we1 = load(d_we1, [F, F], BF16)
            we2 = load(d_we2, [F, F], BF16)
            blkm_b = load(d_blkm_b, [128, 2], BF16)
            blkm_f = load(d_blkm_f, [128, 2], F32)
            wn1 = load(d_wn1, [F, F], F32)
            wn2 = load(d_wn2, [F, F], F32)
            wf2 = load(d_wf2, [F, F], F32)
            ws1 = load(d_ws1, [F, F], F32)
            ws2 = load(d_ws2, [F, F], F32)
            wi1 = load(d_wi1, [F, F], F32)
            wi2 = load(d_wi2, [F, F], F32)
            biasv = load(d_bias, [F, 6], F32)
            bf1bc = load(d_bf1, [128, F + 1], F32) if use_bf1 else None
            bf2row = load(d_bf2, [1, F], F32) if use_bf2 else None

            ident_b = cst.tile([128, 128], BF16)
            make_identity(nc, ident_b[:])
            ident_f = cst.tile([128, 128], F32)
            make_identity(nc, ident_f[:])
            zero64 = cst.tile([128, 64], BF16)
            nc.vector.memset(zero64[:], 0.0)
            nc.sync.dma_start(out=gtab[:, F:4 * F], in_=d_drtab[:, :])

            c_pc = cst.tile([128, NCH], F32)
            cF_pc = cst.tile([128, NCH], F32)
            md_pc = cst.tile([128, 3 * NCH], F32)
            md_blk = cst.tile([128, NCH, 3, 2], BF16)
            m_blk = cst.tile([128, NCH, 2], BF16)
            w6g = cst.tile([128, NCH, 6], BF16)
            staging = cst.tile([128, NCH, 14], F32)   # [G6 | inv2 | dr6]

            # ---------------- node MLP -> imnT, gtab, imn_sb --------------
            p1 = ps.tile([128, 512], F32, tag="ps")
            nc.tensor.matmul(p1[:, 0:A], lhsT=wn1[:], rhs=invT[:], start=True, stop=True)
            h_n = cst.tile([F, A], F32)
            nc.scalar.activation(h_n[:], p1[:, 0:A], ActFn.Silu, bias=biasv[:, 0:1])
            p2 = ps.tile([128, 512], F32, tag="ps")
            nc.tensor.matmul(p2[:, 0:A], lhsT=wn2[:], rhs=h_n[:], start=True, stop=True)
            imnT = cst.tile([F, A], F32)
            nc.scalar.activation(imnT[:], p2[:, 0:A], ActFn.Identity, bias=biasv[:, 1:2])
            imn_rows = cst.tile([128, 2, F], BF16)
            for blk in range(2):
                pt = ps.tile([128, 512], F32, tag="ps")
                nc.tensor.transpose(pt[:, 0:128], imnT[:, blk * 128:(blk + 1) * 128],
                                    ident_f[:])
                nc.scalar.copy(imn_rows[:, blk, :], pt[:, 0:128])
                nc.sync.dma_start(out=gtab[blk * 128:(blk + 1) * 128, 0:F],
                                  in_=imn_rows[:, blk, :])


            # ---------------- cutoff + blocked weights --------------------
            def tt(out, i0, i1, op=AluOp.mult):
                nc.vector.tensor_tensor(out=out, in0=i0, in1=i1, op=op)

            r = cst.tile([128, NCH], F32)
            nc.vector.tensor_scalar_mul(r[:], dist[:], 1.0 / CUTOFF)
            r2 = cst.tile([128, NCH], F32)
            tt(r2[:], r[:], r[:])
            r4 = cst.tile([128, NCH], F32)
            tt(r4[:], r2[:], r2[:])
            r8 = cst.tile([128, NCH], F32)
            tt(r8[:], r4[:], r4[:])
            r9 = cst.tile([128, NCH], F32)
            tt(r9[:], r8[:], r[:])
            k0 = float(-(P_EXP + 1) * (P_EXP + 2) / 2)
            k1 = float(P_EXP * (P_EXP + 2))
            k2 = float(-P_EXP * (P_EXP + 1) / 2)
            t1 = cst.tile([128, NCH], F32)
            nc.vector.tensor_scalar(out=t1[:], in0=r[:], scalar1=k1, scalar2=k0,
                                    op0=AluOp.mult, op1=AluOp.add)
            t2 = cst.tile([128, NCH], F32)
            nc.vector.tensor_scalar_mul(t2[:], r2[:], k2)
            t3 = cst.tile([128, NCH], F32)
            tt(t3[:], t1[:], t2[:], AluOp.add)
            t4 = cst.tile([128, NCH], F32)
            tt(t4[:], r9[:], t3[:])
            lt = cst.tile([128, NCH], F32)
            nc.vector.tensor_scalar(out=lt[:], in0=dist[:], scalar1=CUTOFF,
                                    scalar2=None, op0=AluOp.is_lt)
            t5 = cst.tile([128, NCH], F32)
            nc.vector.tensor_scalar(out=t5[:], in0=t4[:], scalar1=1.0, scalar2=None,
                                    op0=AluOp.add)
            tt(c_pc[:], t5[:], lt[:])

            pc = ps.tile([128, 512], F32, tag="ps")
            nc.tensor.transpose(pc[:, 0:128], c_pc[:], ident_f[:])
            c_rows = cst.tile([128, NCH], BF16)
            nc.vector.tensor_copy(c_rows[:], pc[:, 0:128])

            tt(md_pc[:], dv_pc[:], _ap(m_pc, 0, [[0, 3], [1, NCH]]))
            tt(md_blk[:], _ap(md_pc, 0, [[1, NCH], [NCH, 3], [0, 2]]),
               _ap(blkm_b, 0, [[0, NCH], [0, 3], [1, 2]]))
            tt(m_blk[:], _ap(m_pc, 0, [[1, NCH], [0, 2]]),
               _ap(blkm_b, 0, [[0, NCH], [1, 2]]))

            # ---------------- main edge pipeline --------------------------
            GE = GRP * 128  # edges per group
            for g in range(NG):
                cb = g * GRP
                gath = gathp.tile([128, GRP, 512], BF16, tag="gath")
                # group staging of edge features, imn pairs, c-scaled pattern
                edgeT_g = wrk.tile([NB + 1, GE], BF16, tag="edgeT_g")
                nc.sync.dma_start(out=edgeT_g[:], in_=d_edgeT[:, cb * 128:cb * 128 + GE])
                imn_sb = wrk.tile([2, GE], BF16, tag="imn_sb")
                for a2 in range(2):
                    src = bass.AP(gtab, a2 * 512 + cb * 1024, [[1024, GRP], [1, 128]])
                    nc.sync.dma_start(out=imn_sb[a2:a2 + 1, :], in_=src)
                blk2 = wrk.tile([2, GE], BF16, tag="blk2")

                def bdst(a2, coloff):
                    return bass.AP(blk2.tensor, blk2.offset + a2 * GE + coloff,
                                   [[GE, 1], [128, GRP], [1, 64]])

                nc.sync.dma_start(out=bdst(0, 0), in_=c_rows[cb:cb + GRP, 0:64])
                nc.sync.dma_start(out=bdst(0, 64), in_=zero64[0:GRP, :])
                nc.sync.dma_start(out=bdst(1, 0), in_=zero64[0:GRP, :])
                nc.sync.dma_start(out=bdst(1, 64), in_=c_rows[cb:cb + GRP, 64:128])
                repd = wrk.tile([128, GRP * 128], BF16, tag="repd")
                imed = wrk.tile([128, GRP * 128], BF16, tag="imed")
                ggt = wrk.tile([128, GRP * 128], BF16, tag="ggt")
                imsgR = wrk.tile([128, GRP * 128], BF16, tag="imsgR")
                imsgT = wrk1.tile([128, GRP * 128], BF16, tag="imsgT")
                hfR = wrk1.tile([128, GRP * 128], BF16, tag="hfR")
                heT = wrk1.tile([128, GRP * 128], BF16, tag="heT")
                emdrR = wrk1.tile([128, GRP * 128], BF16, tag="emdrR")
                prods = wrk1.tile([128, 3, GRP * 128], BF16, tag="prods")

                for cl in range(GRP):
                    c = cb + cl
                    nc.gpsimd.indirect_dma_start(
                        out=gath[:, cl, :], out_offset=None, in_=gtab[:],
                        in_offset=IndirectOffsetOnAxis(ap=nbr_sb[:, c:c + 1], axis=0))
                    if cl % 4 == 0:
                        pme = ps.tile([128, 512], F32, tag="ps")
                        prp = ps.tile([128, 512], F32, tag="ps")
                    sl = slice((cl % 4) * 128, (cl % 4) * 128 + 128)
                    nc.tensor.matmul(pme[:, sl],
                                     lhsT=edgeT_g[:, cl * 128:(cl + 1) * 128],
                                     rhs=wme[:], start=True, stop=True)
                    nc.tensor.matmul(prp[:, sl], lhsT=blk2[:, cl * 128:(cl + 1) * 128],
                                     rhs=imn_sb[:, cl * 128:(cl + 1) * 128],
                                     start=True, stop=True)
                    if cl % 4 == 3:
                        dsl = slice((cl - 3) * 128, (cl + 1) * 128)
                        nc.scalar.copy(imed[:, dsl], pme[:])
                        nc.vector.tensor_copy(repd[:, dsl], prp[:])

                tt(ggt[:], _ap(gath, 0, [[512, GRP], [1, 128]]), repd[:])
                tt(imsgR[:], ggt[:], imed[:])

                for cl in range(GRP):
                    if cl % 8 == 0:
                        ptb = ps.tile([128, 1024], BF16, tag="ps")
                    nc.tensor.transpose(ptb[:, (cl % 8) * 128:(cl % 8) * 128 + 128],
                                        imsgR[:, cl * 128:(cl + 1) * 128], ident_b[:])
                    if cl % 8 == 7:
                        dsl = slice((cl - 7) * 128, (cl + 1) * 128)
                        nc.vector.tensor_copy(imsgT[:, dsl], ptb[:])

                for cl in range(GRP):
                    c = cb + cl
                    if cl % 2 == 0:
                        ph = ps.tile([128, 258], F32, tag="ps")
                    po = (cl % 2) * 129
                    nc.tensor.matmul(ph[:, po:po + 129],
                                     lhsT=imsgT[:, cl * 128:(cl + 1) * 128],
                                     rhs=wf1c[:], start=True, stop=True)
                    if cl % 2 == 1:
                        if use_bf1:
                            tt(ph[:], ph[:], _ap(bf1bc, 0, [[0, 2], [1, 129]]),
                               AluOp.add)
                        nc.scalar.activation(hfR[:, (cl - 1) * 128:(cl + 1) * 128],
                                             _ap(ph, 0, [[129, 2], [1, 128]]),
                                             ActFn.Silu)
                        nc.vector.tensor_copy(cF_pc[:, c - 1:c + 1],
                                              _ap(ph, 128, [[129, 2], [1, 1]]))

                for q in range(GRP * 128 // 512):
                    phe = ps.tile([128, 512], F32, tag="ps")
                    nc.tensor.matmul(phe[:], lhsT=we1[:],
                                     rhs=imsgT[:, q * 512:(q + 1) * 512],
                                     start=True, stop=True)
                    nc.scalar.activation(heT[:, q * 512:(q + 1) * 512], phe[:],
                                         ActFn.Silu)

                for cl in range(GRP):
                    if cl % 4 == 0:
                        pem = ps.tile([128, 512], F32, tag="ps")
                    sl = slice((cl % 4) * 128, (cl % 4) * 128 + 128)
                    nc.tensor.matmul(pem[:, sl], lhsT=heT[:, cl * 128:(cl + 1) * 128],
                                     rhs=we2[:], start=True, stop=True)
                    if cl % 4 == 3:
                        dsl = slice((cl - 3) * 128, (cl + 1) * 128)
                        nc.scalar.copy(emdrR[:, dsl], pem[:])

                for d in range(3):
                    tt(prods[:, d, :], emdrR[:],
                       _ap(gath, 128 + d * 128, [[512, GRP], [1, 128]]))

                psk = ps.tile([128, GRP, 14], F32, tag="ps")
                for cl in range(GRP):
                    c = cb + cl
                    nc.vector.tensor_scalar_mul(w6g[:, c, :], md_blk[:, c, :, :],
                                                cF_pc[:, c:c + 1])
                    nc.tensor.matmul(psk[:, cl, 0:6],
                                     lhsT=hfR[:, cl * 128:(cl + 1) * 128],
                                     rhs=w6g[:, c, :], start=True, stop=True)
                    nc.tensor.matmul(psk[:, cl, 6:8],
                                     lhsT=imsgR[:, cl * 128:(cl + 1) * 128],
                                     rhs=m_blk[:, c, :], start=True, stop=True)
                    for d in range(3):
                        nc.tensor.matmul(psk[:, cl, 8 + 2 * d:10 + 2 * d],
                                         lhsT=prods[:, d, cl * 128:(cl + 1) * 128],
                                         rhs=m_blk[:, c, :], start=True, stop=True)
                nc.vector.tensor_copy(staging[:, cb:cb + GRP, :], psk[:])

            # ---------------- atom phase (T-form) -------------------------
            def gslice(base_k):
                """[128, 256] AP over staging: cols (c, a2) at item offset."""
                return _ap(staging, base_k, [[14, NCH], [1, 2]])

            inv_newT = fin.tile([F, A], F32)
            tt(inv_newT[:], invT[:], gslice(6), AluOp.add)

            def node_mlp(w1, w2, b1, b2, src):
                pm = ps.tile([128, 512], F32, tag="ps")
                nc.tensor.matmul(pm[:, 0:A], lhsT=w1[:], rhs=src[:], start=True,
                                 stop=True)
                hh = fin.tile([F, A], F32, name=f"h_{w1.tensor.name}")
                nc.scalar.activation(hh[:], pm[:, 0:A], ActFn.Silu, bias=b1)
                pm2 = ps.tile([128, 512], F32, tag="ps")
                nc.tensor.matmul(pm2[:, 0:A], lhsT=w2[:], rhs=hh[:], start=True,
                                 stop=True)
                out = fin.tile([F, A], F32, name=f"o_{w1.tensor.name}")
                nc.scalar.activation(out[:], pm2[:, 0:A], ActFn.Identity, bias=b2)
                return out

            sT = node_mlp(ws1, ws2, biasv[:, 2:3], biasv[:, 3:4], inv_newT)
            iT = node_mlp(wi1, wi2, biasv[:, 4:5], biasv[:, 5:6], inv_newT)

            # eqf update: Wf2.T @ G_dT (+ bf2 outer S_d)
            eqf_updT = fin.tile([F, 3, A], F32)
            eqf_newT = fin.tile([F, 3, A], F32)
            for d in range(3):
                pe = ps.tile([128, 512], F32, tag="ps")
                nc.tensor.matmul(pe[:, 0:A], lhsT=wf2[:], rhs=gslice(2 * d),
                                 start=True, stop=True)
                nc.vector.tensor_copy(eqf_updT[:, d, :], pe[:, 0:A])
                tt(eqf_newT[:, d, :], _ap(eqfT, 2 * d, [[6, NCH], [1, 2]]),
                   pe[:, 0:A], AluOp.add)

            # eq_F: S[a2,(d,c)] = sum_p blkmask*wcF ; wcF = md*cF
            wcf = fin.tile([128, 3 * NCH], F32)
            tt(wcf[:], md_pc[:], _ap(cF_pc, 0, [[0, 3], [1, NCH]]))
            pS = ps.tile([128, 512], F32, tag="ps")
            nc.tensor.matmul(pS[0:2, 0:3 * NCH], lhsT=blkm_f[:], rhs=wcf[:],
                             start=True, stop=True)
            eqF_new = fin.tile([2, 3 * NCH], F32)
            tt(eqF_new[:], eqFh[:], pS[0:2, 0:3 * NCH], AluOp.add)
            # o_eqF[a, d] with a=2c+a2: flat offset 6c+3a2+d
            nc.sync.dma_start(
                out=bass.AP(o_eqF, 0, [[3, 2], [1, 3], [6, NCH]]),
                in_=bass.AP(eqF_new.tensor, eqF_new.offset,
                            [list(eqF_new.ap[0]), [NCH, 3], [1, NCH]]))

            # eq_dr update: eqdrT + drupdT + sT(broadcast d) * eqf_updT
            tA = fin.tile([F, 3, A], F32)
            for d in range(3):
                tt(tA[:, d, :], _ap(eqdrT, 2 * d, [[6, NCH], [1, 2]]),
                   gslice(8 + 2 * d), AluOp.add)
            eqdr_newT = fin.tile([F, 3, A], F32)
            t_se = fin.tile([F, 3, A], F32)
            tt(t_se[:], _ap(sT, 0, [[0, 3], [1, A]]), eqf_updT[:])
            tt(eqdr_newT[:], tA[:], t_se[:], AluOp.add)

            # inv final: inv_new - iT * sum_d(eqf_new*eqdr_new)
            dotT = fin.tile([F, 3, A], F32)
            tt(dotT[:], eqf_newT[:], eqdr_newT[:])
            dsum = fin.tile([F, A], F32)
            tt(dsum[:], dotT[:, 0, :], dotT[:, 1, :], AluOp.add)
            tt(dsum[:], dsum[:], dotT[:, 2, :], AluOp.add)
            gate = fin.tile([F, A], F32)
            tt(gate[:], iT[:], dsum[:])
            inv_finT = fin.tile([F, A], F32)
            tt(inv_finT[:], inv_newT[:], gate[:], AluOp.subtract)

            # ---------------- outputs (transpose T -> rows) ---------------
            outbuf = fin.tile([128, 128], F32)
            for half in range(2):
                pt = ps.tile([128, 512], F32, tag="ps")
                nc.tensor.transpose(pt[:, 0:128],
                                    inv_finT[:, half * 128:(half + 1) * 128],
                                    ident_f[:])
                nc.vector.tensor_copy(outbuf[:], pt[:, 0:128])
                nc.sync.dma_start(out=o_inv[half * 128:(half + 1) * 128, :],
                                  in_=outbuf[:])
            for (tens, dram) in ((eqf_newT, o_eqf), (eqdr_newT, o_eqdr)):
                for d in range(3):
                    for half in range(2):
                        pt = ps.tile([128, 512], F32, tag="ps")
                        nc.tensor.transpose(pt[:, 0:128],
                                            tens[:, d, half * 128:(half + 1) * 128],
                                            ident_f[:])
                        ob = fin.tile([128, 128], F32,
                                      name=f"ob_{dram.name}_{d}_{half}")
                        nc.vector.tensor_copy(ob[:], pt[:, 0:128])
                        nc.sync.dma_start(
                            out=dram[half * 128:(half + 1) * 128,
                                     d * 128:(d + 1) * 128],
                            in_=ob[:])

    nc.compile()
    return nc


# ======================= host side =======================

_CACHE = {}


def _get_compiled(use_bf1, use_bf2):
    key = (use_bf1, use_bf2)
    if key not in _CACHE:
        import runner_inline as _ri
        nc = build_program(use_bf1, use_bf2)
        _CACHE[key] = _ri.CompiledKernel(nc, n_cores=8)
    return _CACHE[key]


def _marshal_core(b, invariant_node, invariant_edge, distances, distance_vector,
                  neighbors, neighbor_mask, equivariant_node_F, equivariant_node_f,
                  equivariant_node_dr, params):
    f32 = np.float32
    edge = np.asarray(invariant_edge[b], f32).reshape(E, NB)
    edgeT = np.concatenate([edge.T, np.ones((1, E), f32)], 0).astype(BFNP)
    nbr = np.asarray(neighbors[b]).reshape(E).astype(np.int32)
    dist = np.asarray(distances[b], f32).reshape(E)
    mask = np.asarray(neighbor_mask[b], f32).reshape(E)
    dv = np.asarray(distance_vector[b], f32).reshape(E, 3)

    def pc(x):  # [E] -> [128 p, 128 c]
        return np.ascontiguousarray(x.reshape(NCH, 128).T)

    inv = np.asarray(invariant_node[b], f32)            # [A,F]
    eqF = np.asarray(equivariant_node_F[b], f32)        # [A,3]
    eqf = np.asarray(equivariant_node_f[b], f32)        # [A,3,F]
    eqdr = np.asarray(equivariant_node_dr[b], f32)      # [A,3,F]

    # T-form [f, c*6+d*2+a2]
    def t_cda(x):  # x [A,3,F]
        xr = x.reshape(NCH, 2, 3, F)                    # [c,a2,d,f]
        return np.ascontiguousarray(np.transpose(xr, (3, 0, 2, 1)).reshape(F, 6 * NCH))

    eqFh = np.ascontiguousarray(
        np.transpose(eqF.reshape(NCH, 2, 3), (1, 2, 0)).reshape(2, 3 * NCH))

    p = params
    m = {
        "edgeT": edgeT,
        "drtab": np.ascontiguousarray(eqdr.reshape(A, 3 * F)).astype(BFNP),
        "nbr": pc(nbr.astype(f32)).astype(np.int32),
        "dist": pc(dist),
        "mask": pc(mask),
        "dv": np.concatenate([pc(dv[:, d]) for d in range(3)], axis=1),
        "invT": np.ascontiguousarray(inv.T),
        "eqFh": eqFh,
        "eqfT": t_cda(eqf),
        "eqdrT": t_cda(eqdr),
        "wme": np.concatenate([np.asarray(p["Wme"], f32),
                               np.asarray(p["bme"], f32)[None]], 0).astype(BFNP),
        "wf1c": np.concatenate([np.asarray(p["Wf1"], f32),
                                np.asarray(p["Wc"], f32)], 1).astype(BFNP),
        "we1": np.asarray(p["We1"], f32).astype(BFNP),
        "we2": np.asarray(p["We2"], f32).astype(BFNP),
        "blkm_b": np.repeat(np.eye(2, dtype=f32), 64, axis=0).astype(BFNP),
        "blkm_f": np.repeat(np.eye(2, dtype=f32), 64, axis=0),
        "wn1": np.asarray(p["Wn1"], f32), "wn2": np.asarray(p["Wn2"], f32),
        "wf2": np.asarray(p["Wf2"], f32),
        "ws1": np.asarray(p["Ws1"], f32), "ws2": np.asarray(p["Ws2"], f32),
        "wi1": np.asarray(p["Wi1"], f32), "wi2": np.asarray(p["Wi2"], f32),
        "biasv": np.stack([np.asarray(p[k], f32) for k in
                           ("bn1", "bn2", "bs1", "bs2", "bi1", "bi2")], 1),
        "bf1bc": np.tile(np.concatenate([np.asarray(p["bf1"], f32),
                                         np.zeros(1, f32)])[None, :], (128, 1)),
        "bf2row": np.asarray(p["bf2"], f32)[None, :],
    }
    return m


def kernel(invariant_node, invariant_edge, distances, distance_vector,
           neighbors, neighbor_mask, equivariant_node_F, equivariant_node_f,
           equivariant_node_dr, params):
    use_bf1 = bool(np.any(np.asarray(params["bf1"])))
    use_bf2 = bool(np.any(np.asarray(params["bf2"])))
    ck = _get_compiled(use_bf1, use_bf2)
    in_maps = [_marshal_core(b, invariant_node, invariant_edge, distances,
                             distance_vector, neighbors, neighbor_mask,
                             equivariant_node_F, equivariant_node_f,
                             equivariant_node_dr, params) for b in range(B)]
    ck.put(in_maps)
    res = ck.run_np()
    inv = np.stack([r["o_inv"] for r in res])
    eqF = np.stack([r["o_eqF"] for r in res])
    eqf = np.stack([r["o_eqf"] for r in res]).reshape(B, A, 3, F)
    eqdr = np.stack([r["o_eqdr"] for r in res]).reshape(B, A, 3, F)
    if use_bf2:
        # bf2 * S_d correction applied on host is NOT allowed; the device
        # general path is incomplete -- fall back: recompute S_d on device?
        raise NotImplementedError("nonzero bf2 path not supported yet")
    return inv, eqF, eqf, eqdr


# self-contained runner (inlined so kernel.py has no sibling imports)
import sys as _sys
import types as _types

_runner_src = '''
import numpy as np
import jax
from jax.sharding import Mesh, PartitionSpec, NamedSharding
from jax.experimental.shard_map import shard_map
from concourse.bass2jax import _bass_exec_p, install_neuronx_cc_hook, partition_id_tensor
from concourse import mybir


class CompiledKernel:
    def __init__(self, nc, n_cores=8):
        install_neuronx_cc_hook()
        self.nc = nc
        self.n_cores = n_cores
        in_names, out_names, out_avals = [], [], []
        part_name = nc.partition_id_tensor.name if nc.partition_id_tensor else None
        for alloc in nc.m.functions[0].allocations:
            if not isinstance(alloc, mybir.MemoryLocationSet):
                continue
            name = alloc.memorylocations[0].name
            if alloc.kind == "ExternalInput":
                if name != part_name:
                    in_names.append(name)
            elif alloc.kind == "ExternalOutput":
                out_names.append(name)
                out_avals.append(jax.core.ShapedArray(
                    tuple(alloc.tensor_shape), mybir.dt.np(alloc.dtype)))
        self.in_names, self.out_names, self.out_avals = in_names, out_names, out_avals
        all_in = tuple(in_names + out_names + ([part_name] if part_name else []))

        def _body(*args):
            operands = list(args)
            if part_name:
                operands.append(partition_id_tensor())
            return tuple(_bass_exec_p.bind(
                *operands, out_avals=tuple(out_avals), in_names=all_in,
                out_names=tuple(out_names), lowering_input_output_aliases=(),
                sim_require_finite=False, sim_require_nnan=False, nc=nc))

        devices = jax.devices()[:n_cores]
        self.mesh = Mesh(np.asarray(devices), ("core",))
        nin = len(in_names) + len(out_names)
        self.sharding = NamedSharding(self.mesh, PartitionSpec("core"))
        self.f = jax.jit(shard_map(
            _body, mesh=self.mesh, in_specs=(PartitionSpec("core"),) * nin,
            out_specs=(PartitionSpec("core"),) * len(out_names), check_rep=False),
            keep_unused=True)
        self._devin = None

    def put(self, in_maps):
        concat = []
        for name in self.in_names:
            concat.append(np.concatenate([np.asarray(m[name]) for m in in_maps], axis=0))
        for av in self.out_avals:
            concat.append(np.zeros((self.n_cores * av.shape[0],) + av.shape[1:],
                                   av.dtype))
        self._devin = [jax.device_put(a, self.sharding) for a in concat]

    def run(self):
        return self.f(*self._devin)

    def run_np(self):
        outs = self.run()
        jax.block_until_ready(outs)
        res = []
        for c in range(self.n_cores):
            d = {}
            for i, name in enumerate(self.out_names):
                av = self.out_avals[i]
                d[name] = np.asarray(outs[i]).reshape((self.n_cores,) + av.shape)[c]
            res.append(d)
        return res

    def time(self, iters=200, warmup=5):
        import time as _t
        for _ in range(warmup):
            r = self.run()
        jax.block_until_ready(r)
        t0 = _t.perf_counter()
        rs = [self.run() for _ in range(iters)]
        jax.block_until_ready(rs)
        t1 = _t.perf_counter()
        return (t1 - t0) / iters
'''

_mod = _types.ModuleType("runner_inline")
exec(_runner_src, _mod.__dict__)
_sys.modules["runner_inline"] = _mod
